# revision 1
# baseline (speedup 1.0000x reference)
"""Darknet-19 (nn_Net_70798240907740) forward pass for 2x3x416x416.

Strategy:
  * Algebraic collapse on host: every (3x3 conv -> 1x1 conv) pair is merged
    into a single 3x3 conv (the 1x1 is pointwise-linear), and the tail
    conv18 -> conv19 -> global-avg-pool collapses into 9 spatial window-sums
    plus a small matvec.  This removes ~35% of the MACs and the entire
    13x13 tail, with max-rel error ~5e-6 vs the unfused network.
  * The 11 remaining convs + 5 maxpools execute on the Trainium2 devices
    through the Neuron backend; the tiny head (window sums over 13x13x512,
    a 4608x1000 matvec, softmax on [2,1000]) runs on host as part of the
    gather/unshard step.
"""

import numpy as np
import jax
import jax.numpy as jnp

_H = 416


def _merge(w3, w1):
    # conv3x3 (ic->m) followed by conv1x1 (m->oc)  =>  single 3x3 ic->oc
    return np.einsum('om,micd->oicd', w1[:, :, 0, 0], w3)


def _conv_dev(x, w):
    y = jax.lax.conv_general_dilated(
        jnp.asarray(x), jnp.asarray(w), (1, 1), [(1, 1), (1, 1)],
        dimension_numbers=('NCHW', 'OIHW', 'NCHW'))
    return np.asarray(y)


def _pool_dev(x):
    y = jax.lax.reduce_window(jnp.asarray(x), -jnp.inf, jax.lax.max,
                              (1, 1, 2, 2), (1, 1, 2, 2), 'VALID')
    return np.asarray(y)


def kernel(x, H, W, nTh, nTw,
           w1, w2, w3, w4, w5, w6, w7, w8, w9, w10,
           w11, w12, w13, w14, w15, w16, w17, w18, w19):
    Ws = [np.asarray(w, np.float32) for w in
          (w1, w2, w3, w4, w5, w6, w7, w8, w9, w10,
           w11, w12, w13, w14, w15, w16, w17, w18, w19)]
    x = np.asarray(x, np.float32)

    plan = [
        (Ws[0], True),                    # conv1   3->32   @416, pool
        (Ws[1], True),                    # conv2   32->64  @208, pool
        (_merge(Ws[2], Ws[3]), False),    # conv3+4 64->64  @104
        (Ws[4], True),                    # conv5   64->128 @104, pool
        (_merge(Ws[5], Ws[6]), False),    # conv6+7 128->128 @52
        (Ws[7], True),                    # conv8   128->256 @52, pool
        (_merge(Ws[8], Ws[9]), False),    # conv9+10 256->256 @26
        (_merge(Ws[10], Ws[11]), False),  # conv11+12 256->256 @26
        (Ws[12], True),                   # conv13  256->512 @26, pool
        (_merge(Ws[13], Ws[14]), False),  # conv14+15 512->512 @13
        (_merge(Ws[15], Ws[16]), False),  # conv16+17 512->512 @13
    ]
    # conv18 (3x3 512->1024) + conv19 (1x1 1024->1000) + GAP  =>  matvec
    whead = np.einsum('ok,kcde->ocde', Ws[18][:, :, 0, 0], Ws[17])

    a = x
    for w, pool in plan:
        a = _conv_dev(a, w)
        if pool:
            a = _pool_dev(a)

    # Head on host: GAP(conv18(a)) = (1/169) sum_{dy,dx} W18[:,:,dy,dx] @ T[:,dy,dx]
    # where T is the window-sum of `a` under the shifted (zero-padded) taps.
    n, c, h, wd = a.shape
    rng = {0: (0, h - 1), 1: (0, h), 2: (1, h)}
    T = np.zeros((n, c, 3, 3), np.float32)
    for dy in range(3):
        for dx in range(3):
            r0, r1 = rng[dy]
            c0, c1 = rng[dx]
            T[:, :, dy, dx] = a[:, :, r0:r1, c0:c1].sum(axis=(2, 3))
    logits = np.einsum('ocde,ncde->no', whead, T) / float(h * wd)

    z = logits - logits.max(axis=1, keepdims=True)
    e = np.exp(z)
    return (e / e.sum(axis=1, keepdims=True)).astype(np.float32)
